# revision 9
# baseline (speedup 1.0000x reference)
"""Trainium2 Bass kernel for nn_HTM_50354196579134.

Strategy: pure data parallelism over batch (16 rows per core, 8 cores).
All MLP compute (lift, policy pair-scoring, encoder, decoder, classifier,
unlift) runs on-device as compiled Bass/Tile kernels via
run_bass_kernel_spmd. The host replays the (tiny) sequential tree-building
control flow: logits-matrix scatter/masking, softmax/entropy, categorical
sampling via precomputed JAX gumbel bits, and active-set bookkeeping.
"""
import math
import time
import numpy as np

import concourse.bass as bass
import concourse.bacc as bacc
import concourse.mybir as mybir
import concourse.tile as tile
from concourse import bass_utils

F32 = mybir.dt.float32
AF = mybir.ActivationFunctionType

B, N, INP, E = 128, 16, 1024, 512
M = N - 1          # 15 merges
T = N + M          # 31 tree tokens
L = N + M - 1      # 30 logits side
P2 = L * L         # 900
NOISE_STD, MASK_VAL = np.float32(0.01), np.float32(-9e20)
SFTMX_EPS, STD_EPS = 1e-20, 1e-20
NCORES = 8
BC = B // NCORES   # 16 batch rows per core

_KERNELS = {}
_LAUNCH_NS = [0.0]


def _emit_mlp(nc, tc, sb, ps, x_dram, specs, outs, n_cols, chunk):
    """Emit a chain of linear layers over column-blocked transposed input.

    x_dram: DRAM [128, nk0*n_cols] — k-tiles of transposed input, laid out
      so partition p, k-tile kt, column c lives at [p, kt*n_cols + c].
    specs: list of layers; each dict(w=dram [128, nk*dout], b=dram
      [min(128,dout), nmt] or None, relu=bool, out=None or dram target
      ([128, nmt*n_cols] if dout>=128 else [dout, n_cols]), din, dout).
    """
    # load weights/biases to SBUF once; one DMA per k-tile to bound fan-out
    wsb = []
    bsb = []
    for li, sp in enumerate(specs):
        nk = sp["din"] // 128
        dout = sp["dout"]
        wk = []
        wv = sp["w"].ap().rearrange("p (k d) -> p k d", k=nk)
        for kt in range(nk):
            w = sb.tile([128, dout], F32, tag=f"w{li}_{kt}", name=f"w{li}_{kt}")
            nc.gpsimd.dma_start(w[:], wv[:, kt, :])
            wk.append(w)
        wsb.append(wk)
        if sp["b"] is not None:
            nmt = max(1, dout // 128)
            bp = min(128, dout)
            bt = sb.tile([bp, nmt], F32, tag=f"b{li}", name=f"b{li}")
            nc.gpsimd.dma_start(bt[:], sp["b"].ap())
            bsb.append(bt)
        else:
            bsb.append(None)

    nk0 = specs[0]["din"] // 128
    xv = x_dram.ap().rearrange("p (k c) -> p k c", k=nk0)
    nchunk = (n_cols + chunk - 1) // chunk
    for ci in range(nchunk):
        c0 = ci * chunk
        cw = min(chunk, n_cols - c0)
        xk = []
        for kt in range(nk0):
            xt_ = sb.tile([128, chunk], F32, tag=f"x{kt}", name=f"x{kt}")
            nc.gpsimd.dma_start(xt_[:, :cw], xv[:, kt, c0:c0 + cw])
            xk.append(xt_)
        cur = xk      # list of nk sbuf tiles [128, chunk]
        for li, sp in enumerate(specs):
            nk = sp["din"] // 128
            dout = sp["dout"]
            nmt = max(1, dout // 128)
            mp = min(128, dout)
            pst = ps.tile([mp, nmt, chunk], F32, tag=f"ps{li}", name=f"ps{li}")
            for mt in range(nmt):
                for kt in range(nk):
                    lw = wsb[li][kt][:, mt * mp:(mt + 1) * mp]
                    nc.tensor.matmul(
                        pst[:, mt, :cw], lw, cur[kt][:, :cw],
                        start=(kt == 0), stop=(kt == nk - 1),
                    )
            nxt = [sb.tile([mp, chunk], F32, tag=f"h{li}_{mt}", name=f"h{li}_{mt}") for mt in range(nmt)]
            func = AF.Relu if sp["relu"] else AF.Identity
            for mt in range(nmt):
                if bsb[li] is not None:
                    nc.scalar.activation(nxt[mt][:, :cw], pst[:, mt, :cw], func,
                                         bias=bsb[li][:, mt:mt + 1], scale=1.0)
                elif sp["relu"]:
                    nc.scalar.activation(nxt[mt][:, :cw], pst[:, mt, :cw], func)
                else:
                    nc.scalar.copy(nxt[mt][:, :cw], pst[:, mt, :cw])
            if sp.get("out") is not None:
                od = sp["out"]
                if mp == 128:
                    ov = od.ap().rearrange("p (m c) -> p m c", m=nmt)
                    for mt in range(nmt):
                        nc.gpsimd.dma_start(ov[:, mt, c0:c0 + cw], nxt[mt][:, :cw])
                else:
                    nc.gpsimd.dma_start(od.ap()[:, c0:c0 + cw], nxt[0][:, :cw])
            cur = nxt


def _build(name):
    """Build (and cache) one of the six Bass kernels."""
    if name in _KERNELS:
        return _KERNELS[name]
    nc = bacc.Bacc("TRN2", target_bir_lowering=False, debug=False)

    def din(nm, shape):
        return nc.dram_tensor(nm, list(shape), F32, kind="ExternalInput")

    def dout(nm, shape):
        return nc.dram_tensor(nm, list(shape), F32, kind="ExternalOutput")

    with tile.TileContext(nc) as tc:
        with tc.tile_pool(name="sb", bufs=2) as sb, \
             tc.tile_pool(name="ps", bufs=1, space="PSUM") as ps:
            if name == "lift":
                x = din("x", (128, 8 * 256))
                w = din("w", (128, 8 * 512))
                o = dout("o", (128, 4 * 256))
                _emit_mlp(nc, tc, sb, ps, x,
                          [dict(din=1024, dout=512, w=w, b=None, relu=False, out=o)],
                          [o], 256, 256)
            elif name in ("pol0", "pols"):
                ncols = 4096 if name == "pol0" else 960
                x = din("x", (128, 8 * ncols))
                w1 = din("w1", (128, 8 * 512)); b1 = din("b1", (128, 4))
                w2 = din("w2", (128, 4 * 512)); b2 = din("b2", (128, 4))
                w3 = din("w3", (128, 4 * 1))
                o = dout("o", (1, ncols))
                _emit_mlp(nc, tc, sb, ps, x,
                          [dict(din=1024, dout=512, w=w1, b=b1, relu=True),
                           dict(din=512, dout=512, w=w2, b=b2, relu=True),
                           dict(din=512, dout=1, w=w3, b=None, relu=False, out=o)],
                          [o], ncols, 256)
            elif name == "encdec":
                x = din("x", (128, 8 * 16))
                ew1 = din("ew1", (128, 8 * 512)); eb1 = din("eb1", (128, 4))
                ew2 = din("ew2", (128, 4 * 512)); eb2 = din("eb2", (128, 4))
                ew3 = din("ew3", (128, 4 * 512)); eb3 = din("eb3", (128, 4))
                dw1 = din("dw1", (128, 4 * 512)); db1 = din("db1", (128, 4))
                dw2 = din("dw2", (128, 4 * 512)); db2 = din("db2", (128, 4))
                dw3 = din("dw3", (128, 4 * 1024)); db3 = din("db3", (128, 8))
                mg = dout("mg", (128, 4 * 16))
                pred = dout("pred", (128, 8 * 16))
                _emit_mlp(nc, tc, sb, ps, x,
                          [dict(din=1024, dout=512, w=ew1, b=eb1, relu=True),
                           dict(din=512, dout=512, w=ew2, b=eb2, relu=True),
                           dict(din=512, dout=512, w=ew3, b=eb3, relu=False, out=mg),
                           dict(din=512, dout=512, w=dw1, b=db1, relu=True),
                           dict(din=512, dout=512, w=dw2, b=db2, relu=True),
                           dict(din=512, dout=1024, w=dw3, b=db3, relu=False, out=pred)],
                          [mg, pred], 16, 16)
            elif name == "dec":
                x = din("x", (128, 4 * 16))
                dw1 = din("dw1", (128, 4 * 512)); db1 = din("db1", (128, 4))
                dw2 = din("dw2", (128, 4 * 512)); db2 = din("db2", (128, 4))
                dw3 = din("dw3", (128, 4 * 1024)); db3 = din("db3", (128, 8))
                cw1 = din("cw1", (128, 4 * 512)); cb1 = din("cb1", (128, 4))
                cw2 = din("cw2", (128, 4 * 512)); cb2 = din("cb2", (128, 4))
                cw3 = din("cw3", (128, 4 * 2)); cb3 = din("cb3", (2, 1))
                unmg = dout("unmg", (128, 8 * 16))
                clfo = dout("clfo", (2, 16))
                _emit_mlp(nc, tc, sb, ps, x,
                          [dict(din=512, dout=512, w=dw1, b=db1, relu=True),
                           dict(din=512, dout=512, w=dw2, b=db2, relu=True),
                           dict(din=512, dout=1024, w=dw3, b=db3, relu=False, out=unmg)],
                          [unmg], 16, 16)
                _emit_mlp(nc, tc, sb, ps, x,
                          [dict(din=512, dout=512, w=cw1, b=cb1, relu=True),
                           dict(din=512, dout=512, w=cw2, b=cb2, relu=True),
                           dict(din=512, dout=2, w=cw3, b=cb3, relu=False, out=clfo)],
                          [clfo], 16, 16)
            elif name == "fin":
                x = din("x", (128, 4 * 256))
                cw1 = din("cw1", (128, 4 * 512)); cb1 = din("cb1", (128, 4))
                cw2 = din("cw2", (128, 4 * 512)); cb2 = din("cb2", (128, 4))
                cw3 = din("cw3", (128, 4 * 2)); cb3 = din("cb3", (2, 1))
                uw = din("uw", (128, 4 * 1024))
                clfo = dout("clfo", (2, 256))
                recon = dout("recon", (128, 8 * 256))
                _emit_mlp(nc, tc, sb, ps, x,
                          [dict(din=512, dout=512, w=cw1, b=cb1, relu=True),
                           dict(din=512, dout=512, w=cw2, b=cb2, relu=True),
                           dict(din=512, dout=2, w=cw3, b=cb3, relu=False, out=clfo)],
                          [clfo], 256, 256)
                _emit_mlp(nc, tc, sb, ps, x,
                          [dict(din=512, dout=1024, w=uw, b=None, relu=False, out=recon)],
                          [recon], 256, 256)
            else:
                raise ValueError(name)
    if not nc.is_finalized():
        nc.finalize()
    _KERNELS[name] = nc
    return nc


def _launch(name, per_core_ins):
    nc = _build(name)
    t0 = time.time()
    res = bass_utils.run_bass_kernel_spmd(nc, per_core_ins, core_ids=list(range(NCORES)))
    _LAUNCH_NS[0] += (time.time() - t0) * 1e9
    return res.results


def _kt(a, nk):
    """[C, D] activation -> [128, nk*C]: partition p, tile kt, col c."""
    C = a.shape[0]
    return np.ascontiguousarray(
        a.T.reshape(nk, 128, C).transpose(1, 0, 2).reshape(128, nk * C))


def _wkt(w):
    """[din, dout] weight -> [128, (din/128)*dout]."""
    nk = w.shape[0] // 128
    return np.ascontiguousarray(
        w.reshape(nk, 128, -1).transpose(1, 0, 2).reshape(128, -1))


def _bkt(b):
    """[dout] bias -> [min(128,dout), nmt]."""
    d = b.shape[0]
    if d >= 128:
        return np.ascontiguousarray(b.reshape(-1, 128).T)
    return np.ascontiguousarray(b.reshape(1, d).T)


def _from_fm(o, C):
    """[128, nmt*C] feature-major output -> [C, nmt*128]."""
    nmt = o.shape[1] // C
    return np.ascontiguousarray(
        o.reshape(128, nmt, C).transpose(2, 1, 0).reshape(C, nmt * 128))


def _randbits():
    """Precompute all data-independent jax random draws (exact bits, CPU)."""
    import jax
    with jax.default_device(jax.devices("cpu")[0]):
        base = jax.random.key(42)
        lift_noise = NOISE_STD * np.asarray(jax.random.normal(
            jax.random.fold_in(base, 7), (B, N, E), dtype=np.float32))
        gumbel = np.stack([np.asarray(jax.random.gumbel(
            jax.random.fold_in(base, 100 + it), (B, P2), dtype=np.float32))
            for it in range(M)])
        mg_noise = np.stack([NOISE_STD * np.asarray(jax.random.normal(
            jax.random.fold_in(base, 1000 + it), (B, E), dtype=np.float32))
            for it in range(M)])
    return lift_noise, gumbel, mg_noise


def kernel(**inputs):
    inp = {k: np.asarray(v, dtype=np.float32) for k, v in inputs.items()}
    x = inp["x"]
    lift_noise, gumbel, mg_noise = _randbits()

    pol_w = {k: _wkt(inp[f"pol_{k}"]) for k in ("w1", "w2", "w3")}
    pol_b = {k: _bkt(inp[f"pol_{k}"]) for k in ("b1", "b2")}
    pol_b3 = inp["pol_b3"]
    enc_ins = {f"e{k}": _wkt(inp[f"enc_{k}"]) for k in ("w1", "w2", "w3")}
    enc_ins.update({f"e{k}": _bkt(inp[f"enc_{k}"]) for k in ("b1", "b2", "b3")})
    dec_ins = {f"d{k}": _wkt(inp[f"dec_{k}"]) for k in ("w1", "w2", "w3")}
    dec_ins.update({f"d{k}": _bkt(inp[f"dec_{k}"]) for k in ("b1", "b2", "b3")})
    clf_ins = {f"c{k}": _wkt(inp[f"clf_{k}"]) for k in ("w1", "w2", "w3")}
    clf_ins.update({f"c{k}": _bkt(inp[f"clf_{k}"]) for k in ("b1", "b2", "b3")})

    cores = [slice(c * BC, (c + 1) * BC) for c in range(NCORES)]

    # ---- lift ----
    lw = _wkt(inp["lift_w"])
    res = _launch("lift", [dict(x=_kt(x[sl].reshape(BC * N, INP), 8), w=lw)
                           for sl in cores])
    u = np.zeros((B, T, E), dtype=np.float32)
    for c, sl in enumerate(cores):
        u_raw = _from_fm(res[c]["o"], 256).reshape(BC, N, E)
        u[sl, :N] = (u_raw + inp["lift_b"]) + lift_noise[sl]

    # ---- iter 0: all ordered pairs among first N tokens ----
    ii, jj = np.where(~np.eye(N, dtype=bool))
    iif, jjf = np.repeat(np.arange(N), N), np.tile(np.arange(N), N)  # incl diag
    pol0_ins = []
    for sl in cores:
        pf = np.concatenate([u[sl][:, iif], u[sl][:, jjf]], axis=-1)  # [16,256,1024]
        pol0_ins.append(dict(x=_kt(pf.reshape(BC * 256, 2 * E), 8),
                             w1=pol_w["w1"], b1=pol_b["b1"],
                             w2=pol_w["w2"], b2=pol_b["b2"], w3=pol_w["w3"]))
    res = _launch("pol0", pol0_ins)
    logits = np.full((B, L, L), MASK_VAL, dtype=np.float32)
    for c, sl in enumerate(cores):
        lg = (res[c]["o"].reshape(BC, 16, 16) + pol_b3[0]).astype(np.float32)
        logits[sl][:, ii, jj] = lg[:, ii, jj]

    active = np.tile(np.arange(N, dtype=np.int32), (B, 1))
    step_losses = np.zeros(B, dtype=np.float32)
    entropies = np.zeros(B, dtype=np.float32)
    actions, log_probs, rewards = [], [], []
    b_idx = np.arange(B)

    for it in range(M):
        A = active.shape[1]
        if it > 0:
            nt = N + it - 1
            act = active[:, :-1]                     # [B, A-1]
            na = A - 1
            # device pol on padded [2*16*30] columns per core
            pols_ins = []
            for c, sl in enumerate(cores):
                pf = np.zeros((BC, 2, L, 2 * E), dtype=np.float32)
                for bb in range(BC):
                    ua = u[sl.start + bb, act[sl.start + bb]]     # [na, E]
                    un = np.broadcast_to(u[sl.start + bb, nt], (na, E))
                    pf[bb, 0, :na] = np.concatenate([un, ua], -1)
                    pf[bb, 1, :na] = np.concatenate([ua, un], -1)
                pols_ins.append(dict(x=_kt(pf.reshape(BC * 2 * L, 2 * E), 8),
                                     w1=pol_w["w1"], b1=pol_b["b1"],
                                     w2=pol_w["w2"], b2=pol_b["b2"], w3=pol_w["w3"]))
            res = _launch("pols", pols_ins)
            for c, sl in enumerate(cores):
                lg = (res[c]["o"].reshape(BC, 2, L) + pol_b3[0]).astype(np.float32)
                for bb in range(BC):
                    g = sl.start + bb
                    logits[g, nt, act[g]] = lg[bb, 0, :na]
                    logits[g, act[g], nt] = lg[bb, 1, :na]

        flat = logits.reshape(B, P2)
        mx = flat.max(axis=1, keepdims=True)
        ex = np.exp(flat - mx, dtype=np.float32)
        p = (ex / ex.sum(axis=1, keepdims=True)).astype(np.float32)
        p_adj = ((p + np.float32(SFTMX_EPS)) / np.float32(1.0 + P2 * SFTMX_EPS)).astype(np.float32)
        val_norm = SFTMX_EPS / (1.0 + P2 * SFTMX_EPS)
        opt_norm = A * (A - 1)
        nopt = P2 - opt_norm
        S = (p_adj * np.log(p_adj, dtype=np.float32)).sum(axis=1, dtype=np.float32)
        entrop = -(S - np.float32(nopt * (val_norm * math.log(val_norm)))) / np.float32(math.log(opt_norm))
        entropies = (entropies + entrop).astype(np.float32)

        sflat = np.argmax(flat + gumbel[it], axis=1)
        s0, s1 = (sflat // L).astype(np.int32), (sflat % L).astype(np.int32)
        actions.append(np.stack([s0, s1], axis=1))
        log_probs.append(np.log(p[b_idx, sflat], dtype=np.float32))

        for g in range(B):
            logits[g, [s0[g], s1[g]], :] = MASK_VAL
            logits[g, :, [s0[g], s1[g]]] = MASK_VAL

        keep = (active != s0[:, None]) & (active != s1[:, None])
        order = np.argsort((~keep).astype(np.int32), axis=1, kind="stable")
        active = np.take_along_axis(active, order[:, :A - 2], axis=1)

        pair = np.concatenate([u[b_idx, s0], u[b_idx, s1]], axis=1)  # [B, 2E]
        ed_ins = [dict(x=_kt(pair[sl], 8), **enc_ins, **dec_ins) for sl in cores]
        res = _launch("encdec", ed_ins)
        mg = np.zeros((B, E), dtype=np.float32)
        pred = np.zeros((B, 2 * E), dtype=np.float32)
        for c, sl in enumerate(cores):
            mg[sl] = _from_fm(res[c]["mg"], 16)
            pred[sl] = _from_fm(res[c]["pred"], 16)
        osl = ((pred - pair) ** 2).sum(axis=1, dtype=np.float32) / np.float32(2 * E)
        step_losses = (step_losses + osl).astype(np.float32)
        rewards.append(-osl)
        u[:, N + it] = mg + mg_noise[it]
        active = np.concatenate(
            [active, np.full((B, 1), N + it, dtype=np.int32)], axis=1)

    actions = np.stack(actions, axis=1)      # [B, M, 2]
    log_probs = np.stack(log_probs, axis=1).astype(np.float32)
    rewards = np.stack(rewards, axis=1).astype(np.float32)

    # ---- top-down decode ----
    d = np.zeros_like(u)
    for g in range(B):
        d[g, active[g]] = u[g, active[g]]
    clf_rows = []
    for it in range(M):
        tok = T - 1 - it
        mgd = d[:, tok]                       # [B, E]
        dec_in = [dict(x=_kt(mgd[sl], 4), **dec_ins, **clf_ins) for sl in cores]
        res = _launch("dec", dec_in)
        unmg = np.zeros((B, 2 * E), dtype=np.float32)
        crow = np.zeros((B, 2), dtype=np.float32)
        for c, sl in enumerate(cores):
            unmg[sl] = _from_fm(res[c]["unmg"], 16)
            crow[sl] = res[c]["clfo"].T
        un2 = unmg.reshape(B, 2, E)
        a = actions[:, M - 1 - it]
        for g in range(B):
            d[g, a[g, 0]] = un2[g, 0]
            d[g, a[g, 1]] = un2[g, 1]
        clf_rows.append(crow)

    fin_ins = [dict(x=_kt(d[sl, :N].reshape(BC * N, E), 4),
                    **clf_ins, uw=_wkt(inp["unlift_w"])) for sl in cores]
    res = _launch("fin", fin_ins)
    clf2 = np.zeros((B, N, 2), dtype=np.float32)
    recon = np.zeros((B, N, INP), dtype=np.float32)
    for c, sl in enumerate(cores):
        clf2[sl] = res[c]["clfo"].T.reshape(BC, N, 2)
        recon[sl] = (_from_fm(res[c]["recon"], 256).reshape(BC, N, INP)
                     + inp["unlift_b"]).astype(np.float32)

    clf_pred = np.concatenate(
        [np.stack(clf_rows, axis=1), clf2[:, ::-1, :]], axis=1)  # [B, 31, 2]
    clf_lbl = np.tile(np.concatenate(
        [np.ones(M, dtype=np.int32), np.zeros(N, dtype=np.int32)]), (B, 1))

    step_losses = (step_losses / np.float32(M)).astype(np.float32)
    entropies = (entropies / np.float32(M)).astype(np.float32)
    mvg = rewards.reshape(-1)
    rew_n = ((rewards - mvg.mean(dtype=np.float32))
             / (np.std(mvg, ddof=1).astype(np.float32) + np.float32(STD_EPS))).astype(np.float32)
    reinf = (log_probs * rew_n).sum(axis=1, dtype=np.float32)

    return (recon, u, d, step_losses, entropies, clf_pred, clf_lbl, reinf)


def launch_time_ns():
    return _LAUNCH_NS[0]


# revision 11
# speedup vs baseline: 3.9517x; 3.9517x over previous
"""Trainium2 Bass kernel for nn_HTM_50354196579134.

Strategy: pure data parallelism over batch (16 rows per core, 8 cores).
All MLP compute (lift, policy pair-scoring, encoder, decoder, classifier,
unlift) runs on-device as compiled Bass/Tile kernels via
run_bass_kernel_spmd. The host replays the (tiny) sequential tree-building
control flow: logits-matrix scatter/masking, softmax/entropy, categorical
sampling via precomputed JAX gumbel bits, and active-set bookkeeping.
"""
import math
import time
import numpy as np

import concourse.bass as bass
import concourse.bacc as bacc
import concourse.mybir as mybir
import concourse.tile as tile
from concourse import bass_utils

F32 = mybir.dt.float32
AF = mybir.ActivationFunctionType

B, N, INP, E = 128, 16, 1024, 512
M = N - 1          # 15 merges
T = N + M          # 31 tree tokens
L = N + M - 1      # 30 logits side
P2 = L * L         # 900
NOISE_STD, MASK_VAL = np.float32(0.01), np.float32(-9e20)
SFTMX_EPS, STD_EPS = 1e-20, 1e-20
NCORES = 8
BC = B // NCORES   # 16 batch rows per core

_KERNELS = {}
_LAUNCH_NS = [0.0]


def _emit_mlp(nc, tc, sb, ps, x_dram, specs, outs, n_cols, chunk):
    """Emit a chain of linear layers over column-blocked transposed input.

    x_dram: DRAM [128, nk0*n_cols] — k-tiles of transposed input, laid out
      so partition p, k-tile kt, column c lives at [p, kt*n_cols + c].
    specs: list of layers; each dict(w=dram [128, nk*dout], b=dram
      [min(128,dout), nmt] or None, relu=bool, out=None or dram target
      ([128, nmt*n_cols] if dout>=128 else [dout, n_cols]), din, dout).
    """
    # load weights/biases to SBUF once; one DMA per k-tile to bound fan-out
    wsb = []
    bsb = []
    for li, sp in enumerate(specs):
        nk = sp["din"] // 128
        dout = sp["dout"]
        wk = []
        wv = sp["w"].ap().rearrange("p (k d) -> p k d", k=nk)
        for kt in range(nk):
            w = sb.tile([128, dout], F32, tag=f"w{li}_{kt}", name=f"w{li}_{kt}")
            nc.gpsimd.dma_start(w[:], wv[:, kt, :])
            wk.append(w)
        wsb.append(wk)
        if sp["b"] is not None:
            nmt = max(1, dout // 128)
            bp = min(128, dout)
            bt = sb.tile([bp, nmt], F32, tag=f"b{li}", name=f"b{li}")
            nc.gpsimd.dma_start(bt[:], sp["b"].ap())
            bsb.append(bt)
        else:
            bsb.append(None)

    nk0 = specs[0]["din"] // 128
    xv = x_dram.ap().rearrange("p (k c) -> p k c", k=nk0)
    nchunk = (n_cols + chunk - 1) // chunk
    for ci in range(nchunk):
        c0 = ci * chunk
        cw = min(chunk, n_cols - c0)
        xk = []
        for kt in range(nk0):
            xt_ = sb.tile([128, chunk], F32, tag=f"x{kt}", name=f"x{kt}")
            nc.gpsimd.dma_start(xt_[:, :cw], xv[:, kt, c0:c0 + cw])
            xk.append(xt_)
        cur = xk      # list of nk sbuf tiles [128, chunk]
        for li, sp in enumerate(specs):
            nk = sp["din"] // 128
            dout = sp["dout"]
            nmt = max(1, dout // 128)
            mp = min(128, dout)
            pst = ps.tile([mp, nmt, chunk], F32, tag=f"ps{li}", name=f"ps{li}")
            for mt in range(nmt):
                for kt in range(nk):
                    lw = wsb[li][kt][:, mt * mp:(mt + 1) * mp]
                    nc.tensor.matmul(
                        pst[:, mt, :cw], lw, cur[kt][:, :cw],
                        start=(kt == 0), stop=(kt == nk - 1),
                    )
            nxt = [sb.tile([mp, chunk], F32, tag=f"h{li}_{mt}", name=f"h{li}_{mt}") for mt in range(nmt)]
            func = AF.Relu if sp["relu"] else AF.Identity
            for mt in range(nmt):
                if bsb[li] is not None:
                    nc.scalar.activation(nxt[mt][:, :cw], pst[:, mt, :cw], func,
                                         bias=bsb[li][:, mt:mt + 1], scale=1.0)
                elif sp["relu"]:
                    nc.scalar.activation(nxt[mt][:, :cw], pst[:, mt, :cw], func)
                else:
                    nc.scalar.copy(nxt[mt][:, :cw], pst[:, mt, :cw])
            if sp.get("out") is not None:
                od = sp["out"]
                if mp == 128:
                    ov = od.ap().rearrange("p (m c) -> p m c", m=nmt)
                    for mt in range(nmt):
                        nc.gpsimd.dma_start(ov[:, mt, c0:c0 + cw], nxt[mt][:, :cw])
                else:
                    nc.gpsimd.dma_start(od.ap()[:, c0:c0 + cw], nxt[0][:, :cw])
            cur = nxt


def _build(name):
    """Build (and cache) one of the six Bass kernels."""
    if name in _KERNELS:
        return _KERNELS[name]
    nc = bacc.Bacc("TRN2", target_bir_lowering=False, debug=False)

    def din(nm, shape):
        return nc.dram_tensor(nm, list(shape), F32, kind="ExternalInput")

    def dout(nm, shape):
        return nc.dram_tensor(nm, list(shape), F32, kind="ExternalOutput")

    with tile.TileContext(nc) as tc:
        with tc.tile_pool(name="sb", bufs=2) as sb, \
             tc.tile_pool(name="ps", bufs=1, space="PSUM") as ps:
            if name == "lift":
                x = din("x", (128, 8 * 256))
                w = din("w", (128, 8 * 512))
                o = dout("o", (128, 4 * 256))
                _emit_mlp(nc, tc, sb, ps, x,
                          [dict(din=1024, dout=512, w=w, b=None, relu=False, out=o)],
                          [o], 256, 256)
            elif name in ("pol0", "pols"):
                ncols = 4096 if name == "pol0" else 960
                x = din("x", (128, 8 * ncols))
                w1 = din("w1", (128, 8 * 512)); b1 = din("b1", (128, 4))
                w2 = din("w2", (128, 4 * 512)); b2 = din("b2", (128, 4))
                w3 = din("w3", (128, 4 * 1))
                o = dout("o", (1, ncols))
                _emit_mlp(nc, tc, sb, ps, x,
                          [dict(din=1024, dout=512, w=w1, b=b1, relu=True),
                           dict(din=512, dout=512, w=w2, b=b2, relu=True),
                           dict(din=512, dout=1, w=w3, b=None, relu=False, out=o)],
                          [o], ncols, 256)
            elif name == "encdec":
                x = din("x", (128, 8 * 16))
                ew1 = din("ew1", (128, 8 * 512)); eb1 = din("eb1", (128, 4))
                ew2 = din("ew2", (128, 4 * 512)); eb2 = din("eb2", (128, 4))
                ew3 = din("ew3", (128, 4 * 512)); eb3 = din("eb3", (128, 4))
                dw1 = din("dw1", (128, 4 * 512)); db1 = din("db1", (128, 4))
                dw2 = din("dw2", (128, 4 * 512)); db2 = din("db2", (128, 4))
                dw3 = din("dw3", (128, 4 * 1024)); db3 = din("db3", (128, 8))
                mg = dout("mg", (128, 4 * 16))
                pred = dout("pred", (128, 8 * 16))
                _emit_mlp(nc, tc, sb, ps, x,
                          [dict(din=1024, dout=512, w=ew1, b=eb1, relu=True),
                           dict(din=512, dout=512, w=ew2, b=eb2, relu=True),
                           dict(din=512, dout=512, w=ew3, b=eb3, relu=False, out=mg),
                           dict(din=512, dout=512, w=dw1, b=db1, relu=True),
                           dict(din=512, dout=512, w=dw2, b=db2, relu=True),
                           dict(din=512, dout=1024, w=dw3, b=db3, relu=False, out=pred)],
                          [mg, pred], 16, 16)
            elif name == "dec":
                x = din("x", (128, 4 * 16))
                dw1 = din("dw1", (128, 4 * 512)); db1 = din("db1", (128, 4))
                dw2 = din("dw2", (128, 4 * 512)); db2 = din("db2", (128, 4))
                dw3 = din("dw3", (128, 4 * 1024)); db3 = din("db3", (128, 8))
                cw1 = din("cw1", (128, 4 * 512)); cb1 = din("cb1", (128, 4))
                cw2 = din("cw2", (128, 4 * 512)); cb2 = din("cb2", (128, 4))
                cw3 = din("cw3", (128, 4 * 2)); cb3 = din("cb3", (2, 1))
                unmg = dout("unmg", (128, 8 * 16))
                clfo = dout("clfo", (2, 16))
                _emit_mlp(nc, tc, sb, ps, x,
                          [dict(din=512, dout=512, w=dw1, b=db1, relu=True),
                           dict(din=512, dout=512, w=dw2, b=db2, relu=True),
                           dict(din=512, dout=1024, w=dw3, b=db3, relu=False, out=unmg)],
                          [unmg], 16, 16)
                _emit_mlp(nc, tc, sb, ps, x,
                          [dict(din=512, dout=512, w=cw1, b=cb1, relu=True),
                           dict(din=512, dout=512, w=cw2, b=cb2, relu=True),
                           dict(din=512, dout=2, w=cw3, b=cb3, relu=False, out=clfo)],
                          [clfo], 16, 16)
            elif name == "fin":
                x = din("x", (128, 4 * 256))
                cw1 = din("cw1", (128, 4 * 512)); cb1 = din("cb1", (128, 4))
                cw2 = din("cw2", (128, 4 * 512)); cb2 = din("cb2", (128, 4))
                cw3 = din("cw3", (128, 4 * 2)); cb3 = din("cb3", (2, 1))
                uw = din("uw", (128, 4 * 1024))
                clfo = dout("clfo", (2, 256))
                recon = dout("recon", (128, 8 * 256))
                _emit_mlp(nc, tc, sb, ps, x,
                          [dict(din=512, dout=512, w=cw1, b=cb1, relu=True),
                           dict(din=512, dout=512, w=cw2, b=cb2, relu=True),
                           dict(din=512, dout=2, w=cw3, b=cb3, relu=False, out=clfo)],
                          [clfo], 256, 256)
                _emit_mlp(nc, tc, sb, ps, x,
                          [dict(din=512, dout=1024, w=uw, b=None, relu=False, out=recon)],
                          [recon], 256, 256)
            else:
                raise ValueError(name)
    if not nc.is_finalized():
        nc.finalize()
    _KERNELS[name] = nc
    return nc


_JIT = {}
_DEVCACHE = {}


def _get_launcher(name):
    """Build (once) a cached jitted shard_map launcher for kernel `name`."""
    if name in _JIT:
        return _JIT[name]
    import jax
    from jax.experimental.shard_map import shard_map
    from jax.sharding import Mesh, PartitionSpec, NamedSharding
    from concourse import bass2jax

    nc = _build(name)
    bass2jax.install_neuronx_cc_hook()
    partition_name = nc.partition_id_tensor.name if nc.partition_id_tensor else None
    in_names, out_names, out_avals, zero_outs = [], [], [], []
    for alloc in nc.m.functions[0].allocations:
        if not isinstance(alloc, mybir.MemoryLocationSet):
            continue
        nm = alloc.memorylocations[0].name
        if alloc.kind == "ExternalInput":
            if nm != partition_name:
                in_names.append(nm)
        elif alloc.kind == "ExternalOutput":
            out_names.append(nm)
            shape = tuple(alloc.tensor_shape)
            dtype = mybir.dt.np(alloc.dtype)
            out_avals.append(jax.core.ShapedArray(shape, dtype))
            zero_outs.append(np.zeros(shape, dtype))
    n_params = len(in_names)
    n_outs = len(out_avals)
    bind_names = list(in_names) + list(out_names)
    if partition_name is not None:
        bind_names.append(partition_name)
    donate = tuple(range(n_params, n_params + n_outs))

    def _body(*args):
        operands = list(args)
        if partition_name is not None:
            operands.append(bass2jax.partition_id_tensor())
        outs = bass2jax._bass_exec_p.bind(
            *operands,
            out_avals=tuple(out_avals),
            in_names=tuple(bind_names),
            out_names=tuple(out_names),
            lowering_input_output_aliases=(),
            sim_require_finite=True,
            sim_require_nnan=True,
            nc=nc,
        )
        return tuple(outs)

    devices = jax.devices()[:NCORES]
    mesh = Mesh(np.asarray(devices), ("core",))
    in_specs = (PartitionSpec("core"),) * (n_params + n_outs)
    out_specs = (PartitionSpec("core"),) * n_outs
    sharded = jax.jit(
        shard_map(_body, mesh=mesh, in_specs=in_specs, out_specs=out_specs,
                  check_rep=False),
        donate_argnums=donate, keep_unused=True,
    )
    shard1 = NamedSharding(mesh, PartitionSpec("core"))
    _JIT[name] = (sharded, in_names, out_names, out_avals, zero_outs, shard1)
    return _JIT[name]


def _launch(name, per_core_ins, cache_all_but=("x",)):
    """Run kernel on all 8 cores. Inputs whose name is not in
    `cache_all_but` are device-cached after the first launch."""
    import jax
    sharded, in_names, out_names, out_avals, zero_outs, shard1 = _get_launcher(name)
    t0 = time.time()
    args = []
    for nm in in_names:
        key = (name, nm)
        if nm not in cache_all_but and key in _DEVCACHE:
            args.append(_DEVCACHE[key])
            continue
        g = np.concatenate([np.asarray(m[nm]) for m in per_core_ins], axis=0)
        if nm not in cache_all_but:
            arr = jax.device_put(g, shard1)
            _DEVCACHE[key] = arr
            args.append(arr)
        else:
            args.append(g)
    zeros = [np.zeros((NCORES * z.shape[0], *z.shape[1:]), z.dtype)
             for z in zero_outs]
    out_arrs = sharded(*args, *zeros)
    out_np = [np.asarray(o) for o in out_arrs]
    _LAUNCH_NS[0] += (time.time() - t0) * 1e9
    return [
        {nm: out_np[i].reshape(NCORES, *out_avals[i].shape)[c]
         for i, nm in enumerate(out_names)}
        for c in range(NCORES)
    ]


def _kt(a, nk):
    """[C, D] activation -> [128, nk*C]: partition p, tile kt, col c."""
    C = a.shape[0]
    return np.ascontiguousarray(
        a.T.reshape(nk, 128, C).transpose(1, 0, 2).reshape(128, nk * C))


def _wkt(w):
    """[din, dout] weight -> [128, (din/128)*dout]."""
    nk = w.shape[0] // 128
    return np.ascontiguousarray(
        w.reshape(nk, 128, -1).transpose(1, 0, 2).reshape(128, -1))


def _bkt(b):
    """[dout] bias -> [min(128,dout), nmt]."""
    d = b.shape[0]
    if d >= 128:
        return np.ascontiguousarray(b.reshape(-1, 128).T)
    return np.ascontiguousarray(b.reshape(1, d).T)


def _from_fm(o, C):
    """[128, nmt*C] feature-major output -> [C, nmt*128]."""
    nmt = o.shape[1] // C
    return np.ascontiguousarray(
        o.reshape(128, nmt, C).transpose(2, 1, 0).reshape(C, nmt * 128))


def _randbits():
    """Precompute all data-independent jax random draws (exact bits, CPU)."""
    import jax
    with jax.default_device(jax.devices("cpu")[0]):
        base = jax.random.key(42)
        lift_noise = NOISE_STD * np.asarray(jax.random.normal(
            jax.random.fold_in(base, 7), (B, N, E), dtype=np.float32))
        gumbel = np.stack([np.asarray(jax.random.gumbel(
            jax.random.fold_in(base, 100 + it), (B, P2), dtype=np.float32))
            for it in range(M)])
        mg_noise = np.stack([NOISE_STD * np.asarray(jax.random.normal(
            jax.random.fold_in(base, 1000 + it), (B, E), dtype=np.float32))
            for it in range(M)])
    return lift_noise, gumbel, mg_noise


def kernel(**inputs):
    _DEVCACHE.clear()
    _LAUNCH_NS[0] = 0.0
    inp = {k: np.asarray(v, dtype=np.float32) for k, v in inputs.items()}
    x = inp["x"]
    lift_noise, gumbel, mg_noise = _randbits()

    pol_w = {k: _wkt(inp[f"pol_{k}"]) for k in ("w1", "w2", "w3")}
    pol_b = {k: _bkt(inp[f"pol_{k}"]) for k in ("b1", "b2")}
    pol_b3 = inp["pol_b3"]
    enc_ins = {f"e{k}": _wkt(inp[f"enc_{k}"]) for k in ("w1", "w2", "w3")}
    enc_ins.update({f"e{k}": _bkt(inp[f"enc_{k}"]) for k in ("b1", "b2", "b3")})
    dec_ins = {f"d{k}": _wkt(inp[f"dec_{k}"]) for k in ("w1", "w2", "w3")}
    dec_ins.update({f"d{k}": _bkt(inp[f"dec_{k}"]) for k in ("b1", "b2", "b3")})
    clf_ins = {f"c{k}": _wkt(inp[f"clf_{k}"]) for k in ("w1", "w2", "w3")}
    clf_ins.update({f"c{k}": _bkt(inp[f"clf_{k}"]) for k in ("b1", "b2", "b3")})

    cores = [slice(c * BC, (c + 1) * BC) for c in range(NCORES)]

    # ---- lift ----
    lw = _wkt(inp["lift_w"])
    res = _launch("lift", [dict(x=_kt(x[sl].reshape(BC * N, INP), 8), w=lw)
                           for sl in cores])
    u = np.zeros((B, T, E), dtype=np.float32)
    for c, sl in enumerate(cores):
        u_raw = _from_fm(res[c]["o"], 256).reshape(BC, N, E)
        u[sl, :N] = (u_raw + inp["lift_b"]) + lift_noise[sl]

    # ---- iter 0: all ordered pairs among first N tokens ----
    ii, jj = np.where(~np.eye(N, dtype=bool))
    iif, jjf = np.repeat(np.arange(N), N), np.tile(np.arange(N), N)  # incl diag
    pol0_ins = []
    for sl in cores:
        pf = np.concatenate([u[sl][:, iif], u[sl][:, jjf]], axis=-1)  # [16,256,1024]
        pol0_ins.append(dict(x=_kt(pf.reshape(BC * 256, 2 * E), 8),
                             w1=pol_w["w1"], b1=pol_b["b1"],
                             w2=pol_w["w2"], b2=pol_b["b2"], w3=pol_w["w3"]))
    res = _launch("pol0", pol0_ins)
    logits = np.full((B, L, L), MASK_VAL, dtype=np.float32)
    for c, sl in enumerate(cores):
        lg = (res[c]["o"].reshape(BC, 16, 16) + pol_b3[0]).astype(np.float32)
        logits[sl][:, ii, jj] = lg[:, ii, jj]

    active = np.tile(np.arange(N, dtype=np.int32), (B, 1))
    step_losses = np.zeros(B, dtype=np.float32)
    entropies = np.zeros(B, dtype=np.float32)
    actions, log_probs, rewards = [], [], []
    b_idx = np.arange(B)

    for it in range(M):
        A = active.shape[1]
        if it > 0:
            nt = N + it - 1
            act = active[:, :-1]                     # [B, A-1]
            na = A - 1
            # device pol on padded [2*16*30] columns per core
            pols_ins = []
            for c, sl in enumerate(cores):
                pf = np.zeros((BC, 2, L, 2 * E), dtype=np.float32)
                for bb in range(BC):
                    ua = u[sl.start + bb, act[sl.start + bb]]     # [na, E]
                    un = np.broadcast_to(u[sl.start + bb, nt], (na, E))
                    pf[bb, 0, :na] = np.concatenate([un, ua], -1)
                    pf[bb, 1, :na] = np.concatenate([ua, un], -1)
                pols_ins.append(dict(x=_kt(pf.reshape(BC * 2 * L, 2 * E), 8),
                                     w1=pol_w["w1"], b1=pol_b["b1"],
                                     w2=pol_w["w2"], b2=pol_b["b2"], w3=pol_w["w3"]))
            res = _launch("pols", pols_ins)
            for c, sl in enumerate(cores):
                lg = (res[c]["o"].reshape(BC, 2, L) + pol_b3[0]).astype(np.float32)
                for bb in range(BC):
                    g = sl.start + bb
                    logits[g, nt, act[g]] = lg[bb, 0, :na]
                    logits[g, act[g], nt] = lg[bb, 1, :na]

        flat = logits.reshape(B, P2)
        mx = flat.max(axis=1, keepdims=True)
        ex = np.exp(flat - mx, dtype=np.float32)
        p = (ex / ex.sum(axis=1, keepdims=True)).astype(np.float32)
        p_adj = ((p + np.float32(SFTMX_EPS)) / np.float32(1.0 + P2 * SFTMX_EPS)).astype(np.float32)
        val_norm = SFTMX_EPS / (1.0 + P2 * SFTMX_EPS)
        opt_norm = A * (A - 1)
        nopt = P2 - opt_norm
        S = (p_adj * np.log(p_adj, dtype=np.float32)).sum(axis=1, dtype=np.float32)
        entrop = -(S - np.float32(nopt * (val_norm * math.log(val_norm)))) / np.float32(math.log(opt_norm))
        entropies = (entropies + entrop).astype(np.float32)

        sflat = np.argmax(flat + gumbel[it], axis=1)
        s0, s1 = (sflat // L).astype(np.int32), (sflat % L).astype(np.int32)
        actions.append(np.stack([s0, s1], axis=1))
        log_probs.append(np.log(p[b_idx, sflat], dtype=np.float32))

        for g in range(B):
            logits[g, [s0[g], s1[g]], :] = MASK_VAL
            logits[g, :, [s0[g], s1[g]]] = MASK_VAL

        keep = (active != s0[:, None]) & (active != s1[:, None])
        order = np.argsort((~keep).astype(np.int32), axis=1, kind="stable")
        active = np.take_along_axis(active, order[:, :A - 2], axis=1)

        pair = np.concatenate([u[b_idx, s0], u[b_idx, s1]], axis=1)  # [B, 2E]
        ed_ins = [dict(x=_kt(pair[sl], 8), **enc_ins, **dec_ins) for sl in cores]
        res = _launch("encdec", ed_ins)
        mg = np.zeros((B, E), dtype=np.float32)
        pred = np.zeros((B, 2 * E), dtype=np.float32)
        for c, sl in enumerate(cores):
            mg[sl] = _from_fm(res[c]["mg"], 16)
            pred[sl] = _from_fm(res[c]["pred"], 16)
        osl = ((pred - pair) ** 2).sum(axis=1, dtype=np.float32) / np.float32(2 * E)
        step_losses = (step_losses + osl).astype(np.float32)
        rewards.append(-osl)
        u[:, N + it] = mg + mg_noise[it]
        active = np.concatenate(
            [active, np.full((B, 1), N + it, dtype=np.int32)], axis=1)

    actions = np.stack(actions, axis=1)      # [B, M, 2]
    log_probs = np.stack(log_probs, axis=1).astype(np.float32)
    rewards = np.stack(rewards, axis=1).astype(np.float32)

    # ---- top-down decode ----
    d = np.zeros_like(u)
    for g in range(B):
        d[g, active[g]] = u[g, active[g]]
    clf_rows = []
    for it in range(M):
        tok = T - 1 - it
        mgd = d[:, tok]                       # [B, E]
        dec_in = [dict(x=_kt(mgd[sl], 4), **dec_ins, **clf_ins) for sl in cores]
        res = _launch("dec", dec_in)
        unmg = np.zeros((B, 2 * E), dtype=np.float32)
        crow = np.zeros((B, 2), dtype=np.float32)
        for c, sl in enumerate(cores):
            unmg[sl] = _from_fm(res[c]["unmg"], 16)
            crow[sl] = res[c]["clfo"].T
        un2 = unmg.reshape(B, 2, E)
        a = actions[:, M - 1 - it]
        for g in range(B):
            d[g, a[g, 0]] = un2[g, 0]
            d[g, a[g, 1]] = un2[g, 1]
        clf_rows.append(crow)

    fin_ins = [dict(x=_kt(d[sl, :N].reshape(BC * N, E), 4),
                    **clf_ins, uw=_wkt(inp["unlift_w"])) for sl in cores]
    res = _launch("fin", fin_ins)
    clf2 = np.zeros((B, N, 2), dtype=np.float32)
    recon = np.zeros((B, N, INP), dtype=np.float32)
    for c, sl in enumerate(cores):
        clf2[sl] = res[c]["clfo"].T.reshape(BC, N, 2)
        recon[sl] = (_from_fm(res[c]["recon"], 256).reshape(BC, N, INP)
                     + inp["unlift_b"]).astype(np.float32)

    clf_pred = np.concatenate(
        [np.stack(clf_rows, axis=1), clf2[:, ::-1, :]], axis=1)  # [B, 31, 2]
    clf_lbl = np.tile(np.concatenate(
        [np.ones(M, dtype=np.int32), np.zeros(N, dtype=np.int32)]), (B, 1))

    step_losses = (step_losses / np.float32(M)).astype(np.float32)
    entropies = (entropies / np.float32(M)).astype(np.float32)
    mvg = rewards.reshape(-1)
    rew_n = ((rewards - mvg.mean(dtype=np.float32))
             / (np.std(mvg, ddof=1).astype(np.float32) + np.float32(STD_EPS))).astype(np.float32)
    reinf = (log_probs * rew_n).sum(axis=1, dtype=np.float32)

    return (recon, u, d, step_losses, entropies, clf_pred, clf_lbl, reinf)


def launch_time_ns():
    return _LAUNCH_NS[0]
